# revision 38
# baseline (speedup 1.0000x reference)
"""Multi-head attention block on 8 Trainium2 NeuronCores.

Problem: B=8, N=1024, E=768, H=12, D=64 attention (QKV proj -> softmax(QK^T/8)V
-> output proj), fp32 I/O.

Sharding: data parallel over batch - core b computes batch element b entirely
locally; no collectives. Host shards/stacks.

Per-core kernel v3: the PE engine's row count (~311k rows ~= 130us) is the
floor; v3 keeps PE dense the whole way through:
  - tick engine: each S(h,kc) unit (2x512 matmuls + exp on ACT) is paced with
    filler work pulled from a queue (QK/V/proj groups) sized against a rolling
    row budget, so the PE never stalls on the S-psum recycle (exp at ~1.04us
    per tile is slower than the 0.43us S matmul pair).
  - AV(h) q-chunks ride a deque: pushed when exp(h) completes, drained 1-3
    units per tick. AV(10) drains inside the train; only AV(11) trails.
  - output proj is split into partial-sum phases Pa=c0..2 (+bias), Pb=c3,
    Pc=c4, Pd=c5, chained by in-place adds into an fp32 staging tile, so proj
    work streams in as head pairs finish instead of serializing at the end.
  - SWDGE cast-DMAs are consolidated (the 994ns descriptor-gen cost per DMA
    lands on the Pool engine) and the bias columns ride one HWDGE DMA, so
    Pool is free for evac work from ~8us.
  - evacs: front (pre-exp) evacs go to ACT (Copy / Identity+bias-column) and
    DVE; in-train evacs are split DVE/Pool; the fin scales (which gate the
    pa psum recycle) stay on DVE.
"""
import numpy as np

B, N, E, H, D = 8, 1024, 768, 12, 64
SCALE = D ** -0.5
NT = N // 128   # token chunks (8)
NE = E // 128   # embed chunks (6)
NQ = N // 512   # moving-dim tiles (2)
NFS = [(0, 512), (512, 256)]  # free-dim split of E for matmuls


def _build():
    import concourse.bacc as bacc
    import concourse.mybir as mybir
    import concourse.tile as tile
    from concourse.masks import make_identity
    from contextlib import ExitStack

    F32 = mybir.dt.float32
    BF16 = mybir.dt.bfloat16
    EXP = mybir.ActivationFunctionType.Exp
    IDENT = mybir.ActivationFunctionType.Identity
    MULT = mybir.AluOpType.mult
    ADD = mybir.AluOpType.add

    nc = bacc.Bacc("TRN2", target_bir_lowering=False)
    x_d = nc.declare_dram_parameter("x", [N, E], F32, isOutput=False)
    wqkv_d = nc.declare_dram_parameter("W_qkv", [E, 3 * E], F32, isOutput=False)
    bqkv_d = nc.declare_dram_parameter("b_qkv", [3 * E], F32, isOutput=False)
    wproj_d = nc.declare_dram_parameter("W_proj", [E, E], F32, isOutput=False)
    bproj_d = nc.declare_dram_parameter("b_proj", [E], F32, isOutput=False)
    out_d = nc.declare_dram_parameter("out", [N, E], BF16, isOutput=True)

    with tile.TileContext(nc) as tc:
        with (
            tc.tile_pool(name="const", bufs=1) as cp,
            tc.tile_pool(name="qkv", bufs=1) as qp,
            tc.tile_pool(name="psum", bufs=1, space="PSUM") as ps,
            tc.tile_pool(name="proj", bufs=1) as pp,
        ):
            # ---- constants ----
            identf = cp.tile([128, 128], F32)
            make_identity(nc, identf)
            ident_b = cp.tile([128, 128], BF16)
            nc.vector.tensor_copy(ident_b, identf)
            ones1 = cp.tile([1, 128], BF16)
            nc.vector.memset(ones1, 1.0)
            onesH = cp.tile([128, H], BF16)
            nc.vector.memset(onesH, 1.0)
            bq_all = cp.tile([128, 12], F32)

            # ---- long-lived attention-layout tensors ----
            qT = [qp.tile([128, N], BF16, name=f"qT{c}", tag=f"qT{c}")
                  for c in range(6)]
            kT = [qp.tile([128, N], BF16, name=f"kT{c}", tag=f"kT{c}")
                  for c in range(6)]
            vS = [qp.tile([128, 65 * H], BF16, name=f"vS{i}", tag=f"vS{i}")
                  for i in range(NT)]
            attnT = [qp.tile([128, N], BF16, name=f"attnT{p}", tag=f"attnT{p}")
                     for p in range(6)]
            # fp32 staging for the proj partial-sum chain
            o_acc = [pp.tile([128, E], F32, name=f"oacc{i}", tag=f"oacc{i}")
                     for i in range(NT)]

            with ExitStack() as _xs:
                bigp = tc.alloc_tile_pool(name="bigp", bufs=1)
                xtsp = tc.alloc_tile_pool(name="xtsp", bufs=1)

                # ---- DMAs ----
                # SWDGE (gpsimd, casting) DMAs consolidated: each costs 994ns
                # of Pool-engine descriptor generation. bq rides one HWDGE
                # DMA. Issue order = urgency order.
                xts2 = []

                def x_dma(k):
                    xp2 = xtsp.tile([128, 2 * E], BF16, name=f"xt{k}",
                                    tag=f"xt{k}")
                    nc.gpsimd.dma_start(
                        out=xp2.rearrange("p (ii f) -> p ii f", f=E),
                        in_=x_d[k * 256:(k + 1) * 256, :].rearrange(
                            "(ii p) f -> p ii f", p=128))
                    xts2.append(xp2)

                # stage blocks: stage(c)[p, (j f)] holds W columns
                # c*128:(c+1)*128 (Q) / E+c*128.. (K) for all 6 j-chunks
                stq01 = bigp.tile([128, 2 * E], BF16)
                stk01 = bigp.tile([128, 2 * E], BF16)
                stq25 = bigp.tile([128, 4 * E], BF16)
                stk25 = bigp.tile([128, 4 * E], BF16)

                # block layout [p, (j c f)]: the (c f) dims stay adjacent so
                # the DMA APs merge to 3 dims
                def stage_block_dma(dst, col0, ncols):
                    nc.gpsimd.dma_start(
                        out=dst.rearrange("p (j cf) -> p j cf",
                                          cf=ncols * 128),
                        in_=wqkv_d[:, col0:col0 + ncols * 128].rearrange(
                            "(j p) cf -> p j cf", p=128))

                def stage_sl(which, c, j):
                    blk = ((stq01, stq25), (stk01, stk25))[which][c >= 2]
                    nb = 2 if c < 2 else 4
                    cb = c if c < 2 else c - 2
                    o = (j * nb + cb) * 128
                    return blk[:, o:o + 128]

                x_dma(0)
                x_dma(1)
                stage_block_dma(stq01, 0, 2)
                stage_block_dma(stk01, E, 2)
                x_dma(2)
                x_dma(3)
                nc.sync.dma_start(
                    out=bq_all,
                    in_=bqkv_d[0:12 * 128].rearrange("(o p) -> p o", p=128))
                bv_row = bigp.tile([1, E], BF16)
                nc.gpsimd.dma_start(
                    out=bv_row,
                    in_=bqkv_d[2 * E:3 * E].rearrange("(o f) -> o f", o=1))
                wqv_all = bigp.tile([128, NE * E], BF16)
                nc.gpsimd.dma_start(
                    out=wqv_all.rearrange("p (j f) -> p j f", f=E),
                    in_=wqkv_d[:, 2 * E:].rearrange("(j p) f -> p j f",
                                                    p=128))

                def wqv(j):
                    return wqv_all[:, j * E:(j + 1) * E]

                stage_block_dma(stq25, 2 * 128, 4)
                stage_block_dma(stk25, E + 2 * 128, 4)
                wp_all = pp.tile([128, NE * E], BF16)
                nc.gpsimd.dma_start(
                    out=wp_all.rearrange("p (c f) -> p c f", f=E),
                    in_=wproj_d[:, :].rearrange("(c p) f -> p c f", p=128))

                def wp_sb(c):
                    return wp_all[:, c * E:(c + 1) * E]

                bp_row = pp.tile([1, E], BF16)
                nc.gpsimd.dma_start(
                    out=bp_row, in_=bproj_d[:].rearrange("(o f) -> o f", o=1))

                def xchunk(i, j):
                    return xts2[i // 2][:, (i % 2) * E + j * 128:
                                        (i % 2) * E + (j + 1) * 128]

                # ---- engine alternation for in-train evacs ----
                # GPSIMD cannot access PSUM: every psum-reading evac goes
                # to DVE (ACT covers some in the front; Pool gets SBUF-only
                # work like the fin scales)
                _evac_i = [0]

                def evac_engine():
                    _evac_i[0] += 1
                    return nc.vector

                # front-only psum tag rotation (all tags free before train);
                # bufs must match each tag's steady-state declaration
                _p0rot = [("p1", 1), ("pa", 1), ("tp", 1), ("s", 2),
                          ("s", 2)]
                _p0i = [0]

                def p0tag():
                    t = _p0rot[_p0i[0] % len(_p0rot)]
                    _p0i[0] += 1
                    return t

                # ---- phase 0: transpose x -> xT, per (k-pair, j) ----
                xT = [bigp.tile([128, N], BF16, name=f"xT{j}", tag=f"xT{j}")
                      for j in range(NE)]

                _xt_i = [0]

                def emit_xT(k, j):
                    # front-only: evacs alternate ACT (idle until the first
                    # exp) and DVE so the 5-deep psum rotation never gates
                    tg, nb = p0tag()
                    pt = ps.tile([128, 256], BF16, name=f"pt{k}_{j}",
                                 tag=tg, bufs=nb)
                    for di in range(2):
                        nc.tensor.transpose(
                            pt[:, di * 128:(di + 1) * 128],
                            xchunk(k * 2 + di, j), ident_b)
                    _xt_i[0] += 1
                    if _xt_i[0] % 2:
                        nc.scalar.copy(xT[j][:, k * 256:(k + 1) * 256], pt)
                    else:
                        nc.vector.tensor_copy(
                            xT[j][:, k * 256:(k + 1) * 256], pt)

                def emit_QK_group(c, q, which, front=False):
                    # one [128, 512] output group: which=0 -> qT, 1 -> kT
                    dst = (qT[c], kT[c])[which]
                    bqi = which * 6 + c
                    pq = ps.tile([128, 512], F32, name=f"pq{c}_{bqi}_{q}",
                                 tag="p1", bufs=1)
                    for j in range(NE):
                        nc.tensor.matmul(
                            pq,
                            stage_sl(which, c, j),
                            xT[j][:, q * 512:(q + 1) * 512],
                            start=(j == 0), stop=(j == NE - 1))
                    if front:
                        # ACT: out = Identity(in*1 + bias-column)
                        nc.scalar.activation(
                            dst[:, q * 512:(q + 1) * 512], pq, IDENT,
                            bias=bq_all[:, bqi:bqi + 1])
                    else:
                        evac_engine().tensor_scalar_add(
                            dst[:, q * 512:(q + 1) * 512], pq,
                            bq_all[:, bqi:bqi + 1])

                bv_bc = bigp.tile([128, E], BF16)

                def emit_V_bias():
                    for nf, (f0, fw) in enumerate(NFS):
                        pbv = ps.tile([128, 512], F32, name=f"pbv{nf}",
                                      tag="p1", bufs=1)
                        nc.tensor.matmul(pbv[:, :fw], ones1,
                                         bv_row[:, f0:f0 + fw],
                                         start=True, stop=True)
                        nc.vector.tensor_copy(bv_bc[:, f0:f0 + fw],
                                              pbv[:, :fw])

                def emit_V_group(i, nf, eng=None):
                    f0, fw = NFS[nf]
                    if nf == 0:
                        nc.gpsimd.tensor_copy(
                            vS[i].rearrange("p (h c) -> p h c", c=65)
                                [:, :, 64:65],
                            onesH.rearrange("p (h o) -> p h o", o=1))
                    pv = ps.tile([128, 512], F32, name=f"pv{i}_{nf}",
                                 tag="p1", bufs=1)
                    for j in range(NE):
                        nc.tensor.matmul(
                            pv[:, :fw],
                            xT[j][:, i * 128:(i + 1) * 128],
                            wqv(j)[:, f0:f0 + fw],
                            start=(j == 0), stop=(j == NE - 1))
                    nh, h0 = fw // D, f0 // D
                    (eng or evac_engine()).tensor_add(
                        vS[i].rearrange("p (h c) -> p h c", c=65)
                            [:, h0:h0 + nh, 0:64],
                        pv[:, :fw].rearrange("p (h d) -> p h d", d=D),
                        bv_bc[:, f0:f0 + fw].rearrange(
                            "p (h d) -> p h d", d=D))

                bp_bc = pp.tile([128, E], BF16)

                def emit_bp_bc():
                    for nf, (f0, fw) in enumerate(NFS):
                        pbp = ps.tile([128, 512], F32, name=f"pbp{nf}",
                                      tag="p1", bufs=1)
                        nc.tensor.matmul(pbp[:, :fw], ones1,
                                         bp_row[:, f0:f0 + fw],
                                         start=True, stop=True)
                        nc.vector.tensor_copy(bp_bc[:, f0:f0 + fw],
                                              pbp[:, :fw])

                # ---- proj partial-sum phases ----
                # six single-step phases: proj step c streams in as soon as
                # pair c finishes (fin(2c+1)). In the tail all remaining
                # steps of a tile merge into one dense psum group.
                o_sb_of = {}
                nxt_ph = {}  # (i, nf) -> next c to accumulate

                # in the tail (train done) every psum tag is free: rotate
                # proj groups across all of them so mm/evac overlap
                tail_mode = [False]
                _tailrot = [("s", 2), ("s", 2), ("pa", 1), ("tp", 1), ("p1", 1)]
                _tail_i = [0]

                def emit_proj_group(ph, i, nf):
                    cur = nxt_ph.get((i, nf), 0)
                    if ph < cur:
                        return  # already covered by a merged tail group
                    hi = 6 if tail_mode[0] else ph + 1
                    nxt_ph[(i, nf)] = hi
                    f0, fw = NFS[nf]
                    if tail_mode[0]:
                        tg, nb = _tailrot[_tail_i[0] % 5]
                        _tail_i[0] += 1
                    else:
                        tg, nb = "p1", 1
                    po = ps.tile([128, 512], F32, name=f"po{ph}_{i}_{nf}",
                                 tag=tg, bufs=nb)
                    inject_bias = tail_mode[0] and cur == 0
                    if inject_bias:
                        nc.tensor.matmul(po[:, :fw], ones1,
                                         bp_row[:, f0:f0 + fw],
                                         start=True, stop=False)
                    for c in range(cur, hi):
                        nc.tensor.matmul(
                            po[:, :fw],
                            attnT[c][:, i * 128:(i + 1) * 128],
                            wp_sb(c)[:, f0:f0 + fw],
                            start=(c == cur and not inject_bias),
                            stop=(c == hi - 1))
                    acc = o_acc[i][:, f0:f0 + fw]
                    first = bp_bc[:, f0:f0 + fw] if cur == 0 else acc
                    eng = evac_engine()
                    if hi == 6:
                        # final: write bf16 out tile and DMA the half
                        if i not in o_sb_of:
                            o_sb_of[i] = pp.tile([128, E], BF16,
                                                 name=f"o{i}", tag="o",
                                                 bufs=4)
                        o_sb = o_sb_of[i]
                        if inject_bias:
                            _tail_i[0] += 1
                            if _tail_i[0] % 2:
                                nc.scalar.copy(o_sb[:, f0:f0 + fw],
                                               po[:, :fw])
                            else:
                                nc.vector.tensor_copy(o_sb[:, f0:f0 + fw],
                                                      po[:, :fw])
                        else:
                            eng.tensor_add(o_sb[:, f0:f0 + fw], po[:, :fw],
                                           first)
                        nc.sync.dma_start(
                            out=out_d[i * 128:(i + 1) * 128, f0:f0 + fw],
                            in_=o_sb[:, f0:f0 + fw])
                    elif cur == 0:
                        eng.tensor_add(acc, po[:, :fw], first)
                    else:
                        # in-place accumulate: o_acc += po
                        eng.scalar_tensor_tensor(
                            acc, po[:, :fw], 1.0, acc, op0=MULT, op1=ADD)

                # ---- phase 2 helpers (S / exp / AV / fin) ----
                expS_of = {}
                av_state = {}
                attnS_of = {}

                def emit_S_kc(h, kc):
                    c, r0 = h // 2, (h % 2) * 64
                    expS = expS_of[h]
                    pss = ps.tile([128, N], F32, name=f"ps{h}_{kc}",
                                  tag="s", bufs=2)
                    for q in range(NQ):
                        nc.tensor.matmul(
                            pss[:, q * 512:(q + 1) * 512],
                            kT[c][r0:r0 + 64, kc * 128:(kc + 1) * 128],
                            qT[c][r0:r0 + 64, q * 512:(q + 1) * 512],
                            start=True, stop=True)
                    nc.scalar.activation(expS[kc], pss, EXP,
                                         scale=float(SCALE))

                def emit_AV_qc(h, qc, kh):
                    # kc half kh of one AV+Z q-chunk accumulation: the kh=0
                    # units only read vS[0..3], so rides can start before V
                    # is fully emitted
                    if h not in av_state:
                        # AV in cols 0..511, Z in cols 512..519
                        av_state[h] = ps.tile([128, N], F32, name=f"pa{h}",
                                              tag="pa", bufs=1)
                    pa = av_state[h]
                    expS = expS_of[h]
                    k0 = kh * 4
                    for kc in range(k0, k0 + 4):
                        nc.tensor.matmul(
                            pa[:, qc * 64:(qc + 1) * 64],
                            expS[kc][:, qc * 128:(qc + 1) * 128],
                            vS[kc][:, h * 65:h * 65 + 64],
                            start=(kc == 0), stop=(kc == NT - 1))
                    for kc in range(k0, k0 + 4):
                        nc.tensor.matmul(
                            pa[:, 512 + qc:512 + qc + 1],
                            expS[kc][:, qc * 128:(qc + 1) * 128],
                            vS[kc][:, h * 65 + 64:h * 65 + 65],
                            start=(kc == 0), stop=(kc == NT - 1))

                def emit_AV_fin(h):
                    # free the pa psum bank with ONE staging copy (the next
                    # head's AV can then start ~2us earlier); recip/scales
                    # work off the SBUF staging tile at leisure
                    c, r0 = h // 2, (h % 2) * 64
                    pa = av_state.pop(h)
                    del expS_of[h]
                    pst = qp.tile([128, 520], F32, name=f"pst{h}",
                                  tag="past", bufs=1)
                    nc.vector.tensor_copy(pst, pa[:, 0:520])
                    rz = qp.tile([128, 8], F32, name=f"rz{h}", tag="rz",
                                 bufs=2)
                    nc.vector.reciprocal(rz, pst[:, 512:512 + NT])
                    if h % 2 == 0:
                        attnS_of[c] = [
                            qp.tile([128, 128], BF16, name=f"as{c}_{qc}",
                                    tag="attnS", bufs=16)
                            for qc in range(NT)]
                    asb = attnS_of[c]
                    for qc in range(NT):
                        nc.gpsimd.tensor_scalar_mul(
                            asb[qc][:, r0:r0 + 64],
                            pst[:, qc * 64:(qc + 1) * 64],
                            rz[:, qc:qc + 1])
                    if h % 2 == 1:
                        ptp = ps.tile([128, N], BF16, name=f"ptp{c}",
                                      tag="tp", bufs=1)
                        for qc in range(NT):
                            nc.tensor.transpose(
                                ptp[:, qc * 128:(qc + 1) * 128],
                                asb[qc], ident_b)
                        # split evac across DVE and Pool in parallel
                        nc.vector.tensor_copy(attnT[c][:, 0:512],
                                              ptp[:, 0:512])
                        nc.vector.tensor_copy(attnT[c][:, 512:N],
                                              ptp[:, 512:N])
                        del attnS_of[c]

                # ================= scheduler =================
                # filler queue: (rows, gate_pair, emit_fn). gate_pair = index
                # of the attnT pair that must be finished first (-1 = none).
                v_half = [False]
                v_done = [False]

                def emit_V_mark(i, nf):
                    emit_V_group(i, nf)
                    if i == 3 and nf == 1:
                        v_half[0] = True
                    if i == NT - 1 and nf == 1:
                        v_done[0] = True

                filler = []
                for q in range(NQ):
                    for w in range(2):
                        filler.append((512 * NE, -1,
                                       (lambda q=q, w=w:
                                        emit_QK_group(1, q, w))))
                filler.append((768, -1, emit_V_bias))
                for i in range(NT):
                    for nf in (0, 1):
                        filler.append(
                            (NFS[nf][1] * NE, -1,
                             (lambda i=i, nf=nf: emit_V_mark(i, nf))))
                for c in range(2, 6):
                    for q in range(NQ):
                        for w in range(2):
                            filler.append(
                                (512 * NE, -1,
                                 (lambda c=c, q=q, w=w:
                                  emit_QK_group(c, q, w))))
                    if c == 2:
                        filler.append((768, -1, emit_bp_bc))
                # proj step c streams in once pair c is finished
                for ph in range(6):
                    for i in range(NT):
                        for nf in (0, 1):
                            filler.append(
                                (NFS[nf][1], ph,
                                 (lambda ph=ph, i=i, nf=nf:
                                  emit_proj_group(ph, i, nf))))

                fins_done = [-1]   # highest finished pair index

                def pump(budget):
                    # emit filler until `budget` rows spent or nothing ready
                    spent = 0
                    while spent < budget:
                        pick = None
                        for idx, (rows, gate, fn) in enumerate(filler):
                            if gate <= fins_done[0]:
                                pick = idx
                                break
                        if pick is None:
                            break
                        rows = filler[pick][0]
                        if spent + rows / 2 > budget:
                            break
                        rows, gate, fn = filler.pop(pick)
                        fn()
                        spent += rows
                    return spent

                # ---- front ----
                emit_xT(0, 0)
                emit_xT(0, 1)
                emit_xT(1, 0)
                emit_xT(1, 1)
                for j in range(2, NE):
                    emit_xT(0, j)
                    emit_xT(1, j)
                emit_QK_group(0, 0, 0, front=True)
                emit_QK_group(0, 0, 1, front=True)
                for j in range(NE):
                    emit_xT(2, j)
                    emit_xT(3, j)
                emit_QK_group(0, 1, 0, front=True)
                emit_QK_group(0, 1, 1, front=True)
                xtsp.release()

                # ---- train ----
                # AV ride deque: (h, qc, kh) half-units; fin emitted when a
                # head drains. kh=0 halves only need vS[0..3] (v_half).
                ride = []
                TICK = 2500  # target PE rows per tick (~1.04us, exp pace)

                def ride_pop(n):
                    rode = 0
                    for _ in range(n):
                        if not ride:
                            break
                        g, qc, kh = ride[0]
                        if not (v_done[0] if kh else v_half[0]):
                            break
                        ride.pop(0)
                        emit_AV_qc(g, qc, kh)
                        rode += 260
                        if qc == NT - 1 and kh == 1:
                            emit_AV_fin(g)
                            if g % 2 == 1:
                                fins_done[0] = g // 2
                    return rode

                deficit = 0.0
                for h in range(H):
                    expS_of[h] = [
                        qp.tile([128, N], BF16, name=f"expS{h}_{kc}",
                                tag="expS", bufs=28)
                        for kc in range(NT)]
                    for kc in range(NT):
                        emit_S_kc(h, kc)
                        spent = 1024
                        if len(ride) > 32:
                            quota = 8
                        elif len(ride) > 16:
                            quota = 6
                        elif kc < 4:
                            quota = 3
                        else:
                            quota = 2
                        spent += ride_pop(quota)
                        want = TICK + deficit - spent
                        got = pump(max(0, want))
                        # mostly avoid catch-up overfill (it delays the exp
                        # chain), but allow a small positive carry
                        deficit = min(1500.0, max(-3000.0, want - got))
                    ride.extend((h, qc, kh) for kh in (0, 1)
                                for qc in range(NT))
                bigp.release()

                # ---- tail: AV(11) (and any backlog) densely, then the
                # remaining proj groups rotated across all free psum tags
                while ride:
                    ride_pop(len(ride))
                tail_mode[0] = True
                while pump(10 ** 9):
                    pass
    nc.compile()
    return nc


_NC_CACHE = None


def kernel(x, W_qkv, b_qkv, W_proj, b_proj):
    from concourse.bass_utils import run_bass_kernel_spmd

    global _NC_CACHE
    if _NC_CACHE is None:
        _NC_CACHE = _build()
    nc = _NC_CACHE

    x = np.ascontiguousarray(np.asarray(x, dtype=np.float32))
    W_qkv = np.ascontiguousarray(np.asarray(W_qkv, dtype=np.float32))
    b_qkv = np.ascontiguousarray(np.asarray(b_qkv, dtype=np.float32))
    W_proj = np.ascontiguousarray(np.asarray(W_proj, dtype=np.float32))
    b_proj = np.ascontiguousarray(np.asarray(b_proj, dtype=np.float32))

    in_maps = [
        {"x": x[b], "W_qkv": W_qkv, "b_qkv": b_qkv,
         "W_proj": W_proj, "b_proj": b_proj}
        for b in range(B)
    ]
    res = run_bass_kernel_spmd(nc, in_maps, core_ids=list(range(B)))
    return np.stack([np.asarray(res.results[b]["out"]).astype(np.float32)
                     for b in range(B)])
